# revision 15
# baseline (speedup 1.0000x reference)
import numpy as np

N_RAYS = 65536
S = 128
N_CORES = 8
R_CORE = N_RAYS // N_CORES  # 8192

CFG = {
    "plan": (32, 32),
    "relu": (384, 256),
    "full": (0, 1),
    "H": (2, 8),
}

_BUILD_CACHE = {}


def _mkap(T, off, dims):
    from concourse.bass_types import AP
    return AP(T.tensor, T.offset + off, [list(T.ap[0])] + [list(d) for d in dims])


def _squeeze(dims):
    d = [x for x in dims if x[1] != 1]
    return d or [[1, 1]]


def _stage_aps(specs, cur, oth, p, k):
    out = []
    for kind, lo, hi in specs:
        w = hi - lo
        nA = w // (2 * p)
        if k == p:
            dims = _squeeze([[2 * p, nA], [1, p]])
            o0, o1 = lo, lo + p
        else:
            cnt = p // k - 1
            dims = _squeeze([[2 * p, nA], [2 * k, cnt], [1, k]])
            o0, o1 = lo + k, lo + 2 * k
        out.append((kind,
                    _mkap(oth, o0, dims), _mkap(cur, o0, dims),
                    _mkap(cur, o1, dims), _mkap(oth, o1, dims)))
    return out


def build(rpc=R_CORE, plan=None, relu=None, full=None, H=None):
    """Build the Bass program for one core processing rpc rays."""
    import concourse.bass as bass
    import concourse.mybir as mybir
    from concourse import tile

    if plan is None:
        if rpc == R_CORE:
            plan, relu, full = CFG["plan"], CFG["relu"], CFG["full"]
        else:
            n = rpc // (128 * 16)
            plan, relu, full = (16,) * n, (256,) * n, (1,) * n
    H = CFG["H"] if H is None else H
    plan, relu, full = list(plan), list(relu), list(full)
    assert sum(plan) * 128 == rpc

    f32 = mybir.dt.float32
    op = mybir.AluOpType
    act = mybir.ActivationFunctionType
    nrmax = max(plan)

    nc = bass.Bass()
    t_in = nc.dram_tensor("t", [rpc, S], f32, kind="ExternalInput")
    sg_in = nc.dram_tensor("sigma", [rpc, S], f32, kind="ExternalInput")
    c_in = nc.dram_tensor("c", [rpc, 3 * S], f32, kind="ExternalInput")
    wi_out = nc.dram_tensor("wi", [rpc, S], f32, kind="ExternalOutput")
    col_out = nc.dram_tensor("color", [rpc, 3], f32, kind="ExternalOutput")
    dep_out = nc.dram_tensor("depth", [rpc, 1], f32, kind="ExternalOutput")

    stages = []
    p = 1
    while p < S:
        k = p
        while k >= 1:
            stages.append((p, k))
            k //= 2
        p *= 2

    with tile.TileContext(nc) as tc:
        with tc.tile_pool(name="consts", bufs=1) as cpool, \
             tc.tile_pool(name="main", bufs=1) as pool:
            mask = cpool.tile([128, nrmax * S], f32)
            nc.gpsimd.memset(mask, 1.0)
            nc.gpsimd.memset(_mkap(mask, 0, [[S, nrmax], [1, 1]]), 0.0)

            base = 0
            for ti, nray in enumerate(plan):
                row = nray * S
                tile_rays = 128 * nray
                rcols = relu[ti]
                isfull = full[ti]
                assert rcols % 128 == 0 and 0 <= rcols < row
                wA = row - rcols
                if rcols > 0:
                    specs = [("dve", 0, wA), ("relu", wA, row)]
                    cregions = [(0, wA), (wA, row)]
                else:
                    specs = [("dve", 0, row)]
                    cregions = [(0, row)]

                ping = pool.tile([128, row], f32, name=f"ping{ti}")
                pong = pool.tile([128, row], f32, name=f"pong{ti}")
                sg = pool.tile([128, row], f32, name=f"sg{ti}")
                ct = pool.tile([128, 3 * row if isfull else 3 * row // 2],
                               f32, name=f"ct{ti}")
                colr = pool.tile([128, 6 * nray], f32, name=f"colr{ti}")
                dpth = pool.tile([128, nray], f32, name=f"dpth{ti}")

                rows = slice(base, base + tile_rays)
                base += tile_rays
                nc.sync.dma_start(
                    out=ping.rearrange("p (r s) -> p r s", r=nray),
                    in_=t_in[rows, :].rearrange("(p r) s -> p r s", r=nray))
                nc.sync.dma_start(
                    out=sg.rearrange("p (r s) -> p r s", r=nray),
                    in_=sg_in[rows, :].rearrange("(p r) s -> p r s", r=nray))
                ccols = 3 * S if isfull else 3 * S // 2
                nc.sync.dma_start(
                    out=ct.rearrange("p (r s) -> p r s", r=nray),
                    in_=c_in[rows, 0:ccols].rearrange(
                        "(p r) s -> p r s", r=nray))

                # ---- sort (split boundary copies per column region) ----
                cur, oth = ping, pong
                for p_, k_ in stages:
                    for kind, dst0, src0, src1, dst1 in \
                            _stage_aps(specs, cur, oth, p_, k_):
                        if kind == "dve":
                            nc.vector.tensor_tensor(
                                dst0, src0, src1, op=op.min)
                            nc.vector.tensor_tensor(
                                dst1, src0, src1, op=op.max)
                        else:
                            nc.gpsimd.tensor_tensor(
                                dst0, src0, src1, op=op.subtract)
                            nc.scalar.activation(
                                dst1, dst0, func=act.Relu,
                                bias=0.0, scale=1.0)
                            nc.gpsimd.tensor_tensor(
                                dst0, src0, dst1, op=op.subtract)
                            nc.gpsimd.tensor_tensor(
                                dst1, src1, dst1, op=op.add)
                    if k_ < p_:
                        for lo, hi in cregions:
                            cd = _squeeze([
                                [2 * p_, (hi - lo) // (2 * p_)],
                                [2 * p_ - k_, 2], [1, k_]])
                            nc.scalar.activation(
                                _mkap(oth, lo, cd), _mkap(cur, lo, cd),
                                func=act.Copy, bias=0.0, scale=1.0)
                    cur, oth = oth, cur
                assert cur is ping  # 28 stages (even)

                # ---- post-sort, chunked over rays (ts in ping) ----
                Ht = H[ti] if isinstance(H, (list, tuple)) else H
                nh = nray // Ht
                for h in range(Ht):
                    c0 = h * nh * S
                    d127 = [[S, nh], [1, S - 1]]
                    dS1 = [[S, nh], [1, 1]]
                    # dt -> pong ; dt[:,:,S-1] = 0
                    nc.gpsimd.tensor_tensor(
                        _mkap(pong, c0, d127), _mkap(ping, c0 + 1, d127),
                        _mkap(ping, c0, d127), op=op.subtract)
                    nc.gpsimd.memset(_mkap(pong, c0 + S - 1, dS1), 0.0)
                    # sdt = sigma*dt (in-place sg)
                    cw = [[1, nh * S]]
                    nc.gpsimd.tensor_tensor(
                        _mkap(sg, c0, cw), _mkap(sg, c0, cw),
                        _mkap(pong, c0, cw), op=op.mult)
                    # cum -> pong
                    nc.vector.tensor_tensor_scan(
                        out=_mkap(pong, c0, cw), data0=_mkap(mask, 0, cw),
                        data1=_mkap(sg, c0, cw),
                        initial=0.0, op0=op.mult, op1=op.add)
                    # G = exp(-cum) -> sg
                    nc.scalar.activation(
                        _mkap(sg, c0, cw), _mkap(pong, c0, cw),
                        func=act.Exp, bias=0.0, scale=-1.0)
                    # wi -> pong
                    nc.gpsimd.tensor_tensor(
                        _mkap(pong, c0 + 1, d127), _mkap(sg, c0, d127),
                        _mkap(sg, c0 + 1, d127), op=op.subtract)
                    nc.scalar.activation(
                        _mkap(pong, c0, dS1), _mkap(sg, c0, dS1),
                        func=act.Copy, bias=1.0, scale=-1.0)
                    # depth = sum wi*ts
                    nc.gpsimd.tensor_tensor(
                        _mkap(sg, c0, cw), _mkap(pong, c0, cw),
                        _mkap(ping, c0, cw), op=op.mult)
                    nc.vector.tensor_reduce(
                        out=_mkap(dpth, h * nh, [[1, nh]]),
                        in_=_mkap(sg, c0, [[S, nh], [1, S]]),
                        axis=mybir.AxisListType.X, op=op.add)
                    # color (full-ct tiles: chunked with everything else)
                    if isfull:
                        ct4 = _mkap(ct, 3 * c0, [[3 * S, nh], [3, S], [1, 3]])
                        wib = _mkap(pong, c0, [[S, nh], [1, S], [0, 3]])
                        nc.gpsimd.tensor_tensor(ct4, ct4, wib, op=op.mult)
                        nc.vector.tensor_reduce(
                            out=_mkap(colr, 3 * h * nh, [[3, nh], [1, 3]]),
                            in_=_mkap(ct, 3 * c0,
                                      [[3 * S, nh], [1, 3], [3, S]]),
                            axis=mybir.AxisListType.X, op=op.add)

                # ---- color for half-ct tiles: two sample-phases ----
                if not isfull:
                    hs = S // 2
                    hr = 3 * S // 2
                    ct4 = _mkap(ct, 0, [[hr, nray], [3, hs], [1, 3]])
                    wib = _mkap(pong, 0, [[S, nray], [1, hs], [0, 3]])
                    nc.gpsimd.tensor_tensor(ct4, ct4, wib, op=op.mult)
                    nc.vector.tensor_reduce(
                        out=_mkap(colr, 0, [[3, nray], [1, 3]]),
                        in_=_mkap(ct, 0, [[hr, nray], [1, 3], [3, hs]]),
                        axis=mybir.AxisListType.X, op=op.add)
                    nc.sync.dma_start(
                        out=ct.rearrange("p (r s) -> p r s", r=nray),
                        in_=c_in[rows, hr:3 * S].rearrange(
                            "(p r) s -> p r s", r=nray))
                    wib = _mkap(pong, hs, [[S, nray], [1, hs], [0, 3]])
                    nc.gpsimd.tensor_tensor(ct4, ct4, wib, op=op.mult)
                    nc.vector.tensor_reduce(
                        out=_mkap(colr, 3 * nray, [[3, nray], [1, 3]]),
                        in_=_mkap(ct, 0, [[hr, nray], [1, 3], [3, hs]]),
                        axis=mybir.AxisListType.X, op=op.add)
                    nc.gpsimd.tensor_tensor(
                        _mkap(colr, 0, [[1, 3 * nray]]),
                        _mkap(colr, 0, [[1, 3 * nray]]),
                        _mkap(colr, 3 * nray, [[1, 3 * nray]]),
                        op=op.add)

                # ---- stores ----
                nc.sync.dma_start(
                    out=wi_out[rows, :].rearrange("(p r) s -> p r s", r=nray),
                    in_=pong.rearrange("p (r s) -> p r s", r=nray))
                nc.sync.dma_start(
                    out=dep_out[rows, :].rearrange(
                        "(p r) one -> p r one", r=nray),
                    in_=dpth.rearrange("p (r one) -> p r one", one=1))
                nc.sync.dma_start(
                    out=col_out[rows, :].rearrange("(p r) ch -> p r ch", r=nray),
                    in_=_mkap(colr, 0, [[3, nray], [1, 3]]))
    return nc


def _legalize_waits(bj):
    """Split multi-sem waits: this walrus build allows one wait per
    instruction, so hoist extras onto same-engine NoOps just before."""
    import json
    d = json.loads(bj)
    ctr = 0
    for f in d["functions"]:
        for b in f["blocks"]:
            out = []
            for ins in b["instructions"]:
                si = ins.get("sync_info")
                waits = (si or {}).get("on_wait") or []
                if len(waits) > 1:
                    for w in waits[:-1]:
                        ctr += 1
                        out.append({
                            "debug": ins.get("debug", 0),
                            "engine": ins.get("engine"),
                            "ins": [], "outs": [],
                            "name": f"I-lw-{ctr}",
                            "opcode": "NoOp",
                            "sync_info": {"on_update": [], "on_wait": [w]},
                        })
                    si["on_wait"] = [waits[-1]]
                out.append(ins)
            b["instructions"] = out
    return json.dumps(d).encode()


def _finalize(nc):
    bj = _legalize_waits(nc.to_json_bytes())
    nc.to_json_bytes = lambda: bj
    return nc


def _get_nc():
    key = (R_CORE,)
    if key not in _BUILD_CACHE:
        _BUILD_CACHE[key] = _finalize(build(R_CORE))
    return _BUILD_CACHE[key]


def kernel(t, sigma, c):
    from concourse.bass_utils import run_bass_kernel_spmd
    t2 = np.ascontiguousarray(t.reshape(N_RAYS, S).astype(np.float32))
    s2 = np.ascontiguousarray(sigma.reshape(N_RAYS, S).astype(np.float32))
    c2 = np.ascontiguousarray(c.reshape(N_RAYS, 3 * S).astype(np.float32))
    nc = _get_nc()
    in_maps = [
        {"t": t2[k * R_CORE:(k + 1) * R_CORE],
         "sigma": s2[k * R_CORE:(k + 1) * R_CORE],
         "c": c2[k * R_CORE:(k + 1) * R_CORE]}
        for k in range(N_CORES)
    ]
    res = run_bass_kernel_spmd(nc, in_maps, list(range(N_CORES)))
    color = np.concatenate([res.results[k]["color"] for k in range(N_CORES)], 0)
    depth = np.concatenate([res.results[k]["depth"] for k in range(N_CORES)], 0)
    wi = np.concatenate([res.results[k]["wi"] for k in range(N_CORES)], 0)
    return color, depth, wi.reshape(N_RAYS, S, 1)


# revision 18
# speedup vs baseline: 1.1606x; 1.1606x over previous
import numpy as np

N_RAYS = 65536
S = 128
N_CORES = 8
R_CORE = N_RAYS // N_CORES  # 8192

CFG = {
    "plan": (32, 32),
    "relu": (384, 256),
    "full": (0, 1),
    "H": (2, 8),
}

_BUILD_CACHE = {}


def _mkap(T, off, dims):
    from concourse.bass_types import AP
    return AP(T.tensor, T.offset + off, [list(T.ap[0])] + [list(d) for d in dims])


def _squeeze(dims):
    d = [x for x in dims if x[1] != 1]
    return d or [[1, 1]]


def _stage_aps(specs, cur, oth, p, k):
    out = []
    for kind, lo, hi in specs:
        w = hi - lo
        nA = w // (2 * p)
        if k == p:
            dims = _squeeze([[2 * p, nA], [1, p]])
            o0, o1 = lo, lo + p
        else:
            cnt = p // k - 1
            dims = _squeeze([[2 * p, nA], [2 * k, cnt], [1, k]])
            o0, o1 = lo + k, lo + 2 * k
        out.append((kind,
                    _mkap(oth, o0, dims), _mkap(cur, o0, dims),
                    _mkap(cur, o1, dims), _mkap(oth, o1, dims)))
    return out


def build(rpc=R_CORE, plan=None, relu=None, full=None, H=None, hp=False):
    """Build the Bass program for one core processing rpc rays."""
    import concourse.bass as bass
    import concourse.mybir as mybir
    from concourse import tile

    if plan is None:
        if rpc == R_CORE:
            plan, relu, full = CFG["plan"], CFG["relu"], CFG["full"]
        else:
            n = rpc // (128 * 16)
            plan, relu, full = (16,) * n, (256,) * n, (1,) * n
    H = CFG["H"] if H is None else H
    plan, relu, full = list(plan), list(relu), list(full)
    assert sum(plan) * 128 == rpc

    f32 = mybir.dt.float32
    op = mybir.AluOpType
    act = mybir.ActivationFunctionType
    nrmax = max(plan)

    nc = bass.Bass()
    t_in = nc.dram_tensor("t", [rpc, S], f32, kind="ExternalInput")
    sg_in = nc.dram_tensor("sigma", [rpc, S], f32, kind="ExternalInput")
    c_in = nc.dram_tensor("c", [rpc, 3 * S], f32, kind="ExternalInput")
    wi_out = nc.dram_tensor("wi", [rpc, S], f32, kind="ExternalOutput")
    col_out = nc.dram_tensor("color", [rpc, 3], f32, kind="ExternalOutput")
    dep_out = nc.dram_tensor("depth", [rpc, 1], f32, kind="ExternalOutput")

    stages = []
    p = 1
    while p < S:
        k = p
        while k >= 1:
            stages.append((p, k))
            k //= 2
        p *= 2

    with tile.TileContext(nc) as tc:
        with tc.tile_pool(name="consts", bufs=1) as cpool, \
             tc.tile_pool(name="main", bufs=1) as pool:
            mask = cpool.tile([128, nrmax * S], f32)
            nc.gpsimd.memset(mask, 1.0)
            nc.gpsimd.memset(_mkap(mask, 0, [[S, nrmax], [1, 1]]), 0.0)

            base = 0
            for ti, nray in enumerate(plan):
                row = nray * S
                tile_rays = 128 * nray
                rcols = relu[ti]
                isfull = full[ti]
                assert rcols % 128 == 0 and 0 <= rcols < row
                wA = row - rcols
                if rcols > 0:
                    specs = [("dve", 0, wA), ("relu", wA, row)]
                    cregions = [(0, wA), (wA, row)]
                else:
                    specs = [("dve", 0, row)]
                    cregions = [(0, row)]

                ping = pool.tile([128, row], f32, name=f"ping{ti}")
                pong = pool.tile([128, row], f32, name=f"pong{ti}")
                sg = pool.tile([128, row], f32, name=f"sg{ti}")
                ct = pool.tile([128, 3 * row if isfull else 3 * row // 2],
                               f32, name=f"ct{ti}")
                colr = pool.tile([128, 6 * nray], f32, name=f"colr{ti}")
                dpth = pool.tile([128, nray], f32, name=f"dpth{ti}")

                rows = slice(base, base + tile_rays)
                base += tile_rays
                nc.sync.dma_start(
                    out=ping.rearrange("p (r s) -> p r s", r=nray),
                    in_=t_in[rows, :].rearrange("(p r) s -> p r s", r=nray))
                nc.sync.dma_start(
                    out=sg.rearrange("p (r s) -> p r s", r=nray),
                    in_=sg_in[rows, :].rearrange("(p r) s -> p r s", r=nray))
                ccols = 3 * S if isfull else 3 * S // 2
                nc.sync.dma_start(
                    out=ct.rearrange("p (r s) -> p r s", r=nray),
                    in_=c_in[rows, 0:ccols].rearrange(
                        "(p r) s -> p r s", r=nray))

                # ---- sort (split boundary copies per column region) ----
                cur, oth = ping, pong
                for p_, k_ in stages:
                    for kind, dst0, src0, src1, dst1 in \
                            _stage_aps(specs, cur, oth, p_, k_):
                        if kind == "dve":
                            nc.vector.tensor_tensor(
                                dst0, src0, src1, op=op.min)
                            nc.vector.tensor_tensor(
                                dst1, src0, src1, op=op.max)
                        else:
                            nc.gpsimd.tensor_tensor(
                                dst0, src0, src1, op=op.subtract)
                            nc.scalar.activation(
                                dst1, dst0, func=act.Relu,
                                bias=0.0, scale=1.0)
                            nc.gpsimd.tensor_tensor(
                                dst0, src0, dst1, op=op.subtract)
                            nc.gpsimd.tensor_tensor(
                                dst1, src1, dst1, op=op.add)
                    if k_ < p_:
                        for lo, hi in cregions:
                            cd = _squeeze([
                                [2 * p_, (hi - lo) // (2 * p_)],
                                [2 * p_ - k_, 2], [1, k_]])
                            import contextlib
                            hctx = (tc.high_priority() if hp
                                    else contextlib.nullcontext())
                            with hctx:
                                nc.scalar.activation(
                                    _mkap(oth, lo, cd), _mkap(cur, lo, cd),
                                    func=act.Copy, bias=0.0, scale=1.0)
                    cur, oth = oth, cur
                assert cur is ping  # 28 stages (even)

                # ---- post-sort, chunked over rays (ts in ping) ----
                Ht = H[ti] if isinstance(H, (list, tuple)) else H
                nh = nray // Ht
                for h in range(Ht):
                    c0 = h * nh * S
                    d127 = [[S, nh], [1, S - 1]]
                    dS1 = [[S, nh], [1, 1]]
                    # dt -> pong ; dt[:,:,S-1] = 0
                    nc.gpsimd.tensor_tensor(
                        _mkap(pong, c0, d127), _mkap(ping, c0 + 1, d127),
                        _mkap(ping, c0, d127), op=op.subtract)
                    nc.gpsimd.memset(_mkap(pong, c0 + S - 1, dS1), 0.0)
                    # sdt = sigma*dt (in-place sg)
                    cw = [[1, nh * S]]
                    nc.gpsimd.tensor_tensor(
                        _mkap(sg, c0, cw), _mkap(sg, c0, cw),
                        _mkap(pong, c0, cw), op=op.mult)
                    # cum -> pong
                    nc.vector.tensor_tensor_scan(
                        out=_mkap(pong, c0, cw), data0=_mkap(mask, 0, cw),
                        data1=_mkap(sg, c0, cw),
                        initial=0.0, op0=op.mult, op1=op.add)
                    # G = exp(-cum) -> sg
                    nc.scalar.activation(
                        _mkap(sg, c0, cw), _mkap(pong, c0, cw),
                        func=act.Exp, bias=0.0, scale=-1.0)
                    # wi -> pong
                    nc.gpsimd.tensor_tensor(
                        _mkap(pong, c0 + 1, d127), _mkap(sg, c0, d127),
                        _mkap(sg, c0 + 1, d127), op=op.subtract)
                    nc.scalar.activation(
                        _mkap(pong, c0, dS1), _mkap(sg, c0, dS1),
                        func=act.Copy, bias=1.0, scale=-1.0)
                    # depth = sum wi*ts
                    nc.gpsimd.tensor_tensor(
                        _mkap(sg, c0, cw), _mkap(pong, c0, cw),
                        _mkap(ping, c0, cw), op=op.mult)
                    nc.vector.tensor_reduce(
                        out=_mkap(dpth, h * nh, [[1, nh]]),
                        in_=_mkap(sg, c0, [[S, nh], [1, S]]),
                        axis=mybir.AxisListType.X, op=op.add)
                    # color (full-ct tiles: chunked with everything else)
                    if isfull:
                        ct4 = _mkap(ct, 3 * c0, [[3 * S, nh], [3, S], [1, 3]])
                        wib = _mkap(pong, c0, [[S, nh], [1, S], [0, 3]])
                        nc.gpsimd.tensor_tensor(ct4, ct4, wib, op=op.mult)
                        nc.vector.tensor_reduce(
                            out=_mkap(colr, 3 * h * nh, [[3, nh], [1, 3]]),
                            in_=_mkap(ct, 3 * c0,
                                      [[3 * S, nh], [1, 3], [3, S]]),
                            axis=mybir.AxisListType.X, op=op.add)

                # ---- color for half-ct tiles: two sample-phases ----
                if not isfull:
                    hs = S // 2
                    hr = 3 * S // 2
                    ct4 = _mkap(ct, 0, [[hr, nray], [3, hs], [1, 3]])
                    wib = _mkap(pong, 0, [[S, nray], [1, hs], [0, 3]])
                    nc.gpsimd.tensor_tensor(ct4, ct4, wib, op=op.mult)
                    nc.vector.tensor_reduce(
                        out=_mkap(colr, 0, [[3, nray], [1, 3]]),
                        in_=_mkap(ct, 0, [[hr, nray], [1, 3], [3, hs]]),
                        axis=mybir.AxisListType.X, op=op.add)
                    nc.sync.dma_start(
                        out=ct.rearrange("p (r s) -> p r s", r=nray),
                        in_=c_in[rows, hr:3 * S].rearrange(
                            "(p r) s -> p r s", r=nray))
                    wib = _mkap(pong, hs, [[S, nray], [1, hs], [0, 3]])
                    nc.gpsimd.tensor_tensor(ct4, ct4, wib, op=op.mult)
                    nc.vector.tensor_reduce(
                        out=_mkap(colr, 3 * nray, [[3, nray], [1, 3]]),
                        in_=_mkap(ct, 0, [[hr, nray], [1, 3], [3, hs]]),
                        axis=mybir.AxisListType.X, op=op.add)
                    nc.gpsimd.tensor_tensor(
                        _mkap(colr, 0, [[1, 3 * nray]]),
                        _mkap(colr, 0, [[1, 3 * nray]]),
                        _mkap(colr, 3 * nray, [[1, 3 * nray]]),
                        op=op.add)

                # ---- stores ----
                nc.sync.dma_start(
                    out=wi_out[rows, :].rearrange("(p r) s -> p r s", r=nray),
                    in_=pong.rearrange("p (r s) -> p r s", r=nray))
                nc.sync.dma_start(
                    out=dep_out[rows, :].rearrange(
                        "(p r) one -> p r one", r=nray),
                    in_=dpth.rearrange("p (r one) -> p r one", one=1))
                nc.sync.dma_start(
                    out=col_out[rows, :].rearrange("(p r) ch -> p r ch", r=nray),
                    in_=_mkap(colr, 0, [[3, nray], [1, 3]]))
    return nc


def _legalize_waits(bj):
    """Split multi-sem waits: this walrus build allows one wait per
    instruction, so hoist extras onto same-engine NoOps just before."""
    import json
    d = json.loads(bj)
    ctr = 0
    for f in d["functions"]:
        for b in f["blocks"]:
            out = []
            for ins in b["instructions"]:
                si = ins.get("sync_info")
                waits = (si or {}).get("on_wait") or []
                if len(waits) > 1:
                    for w in waits[:-1]:
                        ctr += 1
                        out.append({
                            "debug": ins.get("debug", 0),
                            "engine": ins.get("engine"),
                            "ins": [], "outs": [],
                            "name": f"I-lw-{ctr}",
                            "opcode": "NoOp",
                            "sync_info": {"on_update": [], "on_wait": [w]},
                        })
                    si["on_wait"] = [waits[-1]]
                out.append(ins)
            b["instructions"] = out
    return json.dumps(d).encode()


def _finalize(nc):
    bj = _legalize_waits(nc.to_json_bytes())
    nc.to_json_bytes = lambda: bj
    return nc


def _get_nc():
    key = (R_CORE,)
    if key not in _BUILD_CACHE:
        _BUILD_CACHE[key] = _finalize(build(R_CORE))
    return _BUILD_CACHE[key]


def kernel(t, sigma, c):
    from concourse.bass_utils import run_bass_kernel_spmd
    t2 = np.ascontiguousarray(t.reshape(N_RAYS, S).astype(np.float32))
    s2 = np.ascontiguousarray(sigma.reshape(N_RAYS, S).astype(np.float32))
    c2 = np.ascontiguousarray(c.reshape(N_RAYS, 3 * S).astype(np.float32))
    nc = _get_nc()
    in_maps = [
        {"t": t2[k * R_CORE:(k + 1) * R_CORE],
         "sigma": s2[k * R_CORE:(k + 1) * R_CORE],
         "c": c2[k * R_CORE:(k + 1) * R_CORE]}
        for k in range(N_CORES)
    ]
    res = run_bass_kernel_spmd(nc, in_maps, list(range(N_CORES)))
    color = np.concatenate([res.results[k]["color"] for k in range(N_CORES)], 0)
    depth = np.concatenate([res.results[k]["depth"] for k in range(N_CORES)], 0)
    wi = np.concatenate([res.results[k]["wi"] for k in range(N_CORES)], 0)
    return color, depth, wi.reshape(N_RAYS, S, 1)
